# revision 2
# baseline (speedup 1.0000x reference)
"""Mixtral expert-capacity top-2 MLP (per-expert grouped GEMM SwiGLU) on 8
NeuronCores: all matmuls in bf16 (FWL weight loads), bf16 hidden
staging, deep-buffered pass 2, HWDGE queue spreading, batched hidden writes.

Expert parallel: core e computes, for expert e,
    out_e = (silu(X_e @ W1_e) * (X_e @ W3_e)) @ W2_e
with X_e = dispatch_input[e] reshaped to (B*C, H).

Structure per core:
  phase 0: transpose X (T,H) -> XT (H,T) via PE-transpose (fp32), DVE/ACT
           evacuate PSUM with cast to bf16; XT bf16 SBUF-resident.
  pass 1:  hidden[f, t] = silu(W1.T X.T) * (W3.T X.T) in bf16 matmuls
           (weights DMA'd fp32, cast to bf16 on DVE/ACT), PSUM accumulate
           over H; hidden staged to DRAM as bf16.
  pass 2:  out[t, h] = hidden.T @ W2 (both bf16, W2 cast on ACT) with PSUM
           accumulation over F in blocks of KFB f-tiles, partials summed in
           an SBUF fp32 accumulator (one H-half at a time), then DMA'd out.

The observe() trick from V1 is kept: every DMA- or engine-produced tile PE
consumes is first "observed" by a throwaway bf16 ldweights so real matmuls
only wait on the PSUM-slot-release semaphore.
"""

import os

os.environ.setdefault("MYCRO_LOCAL_CACHE", "1")

import numpy as np

E, B, C, H, F = 8, 4, 512, 2048, 7168
P = 128

TRACE = bool(int(os.environ.get("BASS_KERNEL_TRACE", "0")))
LAST_RESULT = None
_built = {}


def _build(T, Hd, Fd):
    import concourse.bass as bass
    import concourse.mybir as mybir
    from concourse import bacc
    import concourse.tile as tile
    from concourse.masks import make_identity

    f32 = mybir.dt.float32
    bf16 = mybir.dt.bfloat16
    Silu = mybir.ActivationFunctionType.Silu
    Copy = mybir.ActivationFunctionType.Copy

    assert T % P == 0 and Hd % P == 0 and Fd % P == 0
    NT = T // P          # token tiles
    NH = Hd // P         # H contraction tiles
    NF = Fd // P         # F tiles
    TCH = min(512, T)    # pass-1 moving-dim chunk (tokens)
    NTC = T // TCH
    KFB = 8 if NF % 8 == 0 else 1   # pass-2 f-tiles per PSUM accumulation block
    NKFB = NF // KFB
    NHF = 2              # pass-2 H halves (SBUF accumulator covers T x Hd/NHF)
    assert Hd % NHF == 0
    HH = Hd // NHF
    NQ = min(512, HH)    # pass-2 moving-dim chunk (H)
    NNQ = HH // NQ

    nc = bacc.Bacc(None, target_bir_lowering=False)
    x = nc.declare_dram_parameter("x", [T, Hd], f32, isOutput=False)
    w1 = nc.declare_dram_parameter("w1", [Hd, Fd], f32, isOutput=False)
    w2 = nc.declare_dram_parameter("w2", [Fd, Hd], f32, isOutput=False)
    w3 = nc.declare_dram_parameter("w3", [Hd, Fd], f32, isOutput=False)
    out = nc.declare_dram_parameter("out", [T, Hd], f32, isOutput=True)
    hid_dram = nc.dram_tensor("hid", [Fd, T], bf16)

    w1r = w1.rearrange("(ho hi) f -> hi ho f", hi=P)   # [128, NH, Fd]
    w3r = w3.rearrange("(ho hi) f -> hi ho f", hi=P)

    with tile.TileContext(nc) as tc:
        with tc.tile_pool(name="const", bufs=1) as const_pool:
            ident = const_pool.tile([P, P], f32)
            make_identity(nc, ident)
            identb = const_pool.tile([P, P], bf16)
            nc.vector.tensor_copy(out=identb, in_=ident)

            def observe(t2d):
                """Absorb a wait into PE's clock: a throwaway bf16 ldweights
                reading only this tile.  No output -> no WAW self-wait."""
                nc.tensor.ldweights(t2d[:, 0:64].bitcast(bf16))

            with tc.tile_pool(name="xt", bufs=1) as xt_pool:
                xt = xt_pool.tile([P, NH, T], bf16)  # XT[hi, ho, t]

                # ---- phase 0: X -> XT via PE transpose, cast to bf16 ----
                with tc.tile_pool(name="xload", bufs=2) as xload_pool, \
                     tc.tile_pool(name="pst", bufs=4, space="PSUM") as pst_pool:
                    for tt in range(NT):
                        x_t = xload_pool.tile([P, Hd], f32, tag="xld")
                        nc.sync.dma_start(out=x_t, in_=x[tt * P:(tt + 1) * P, :])
                        observe(x_t[:, 0:P])
                        for ho in range(NH):
                            pt = pst_pool.tile([P, P], f32, tag="pst")
                            nc.tensor.transpose(pt, x_t[:, ho * P:(ho + 1) * P], ident)
                            osl = xt[:, ho, tt * P:(tt + 1) * P]
                            if ho % 2 == 0:
                                nc.vector.tensor_copy(out=osl, in_=pt)
                            else:
                                nc.scalar.activation(out=osl, in_=pt, func=Copy)

                tc.strict_bb_all_engine_barrier()
                observe(identb)  # re-observe post-barrier on a dep-free tile

                # ---- pass 1: hidden = silu(X@W1) * (X@W3), layout [f, t] ----
                with tc.tile_pool(name="w1c", bufs=2) as w1_pool, \
                     tc.tile_pool(name="w3c", bufs=2) as w3_pool, \
                     tc.tile_pool(name="w1b", bufs=2) as w1b_pool, \
                     tc.tile_pool(name="w3b", bufs=2) as w3b_pool, \
                     tc.tile_pool(name="hidst", bufs=4) as hid_pool, \
                     tc.tile_pool(name="ps_mm", bufs=3, space="PSUM") as ps_pool:
                    for ft in range(NF):
                        w1c = w1_pool.tile([P, NH, P], f32, tag="w1c")
                        nc.sync.dma_start(
                            out=w1c, in_=w1r[:, :, ft * P:(ft + 1) * P]
                        )
                        w1b = w1b_pool.tile([P, NH, P], bf16, tag="w1b")
                        nc.vector.tensor_copy(out=w1b, in_=w1c)
                        observe(w1b[:, 0, :])
                        w3c = w3_pool.tile([P, NH, P], f32, tag="w3c")
                        nc.scalar.dma_start(
                            out=w3c, in_=w3r[:, :, ft * P:(ft + 1) * P]
                        )
                        w3b = w3b_pool.tile([P, NH, P], bf16, tag="w3b")
                        nc.scalar.activation(out=w3b, in_=w3c, func=Copy)
                        observe(w3b[:, 0, :])
                        hb = hid_pool.tile([P, T], bf16, tag="hid")
                        for tch in range(NTC):
                            tsl = slice(tch * TCH, (tch + 1) * TCH)
                            ps1 = ps_pool.tile([P, TCH], f32, tag="ps1")
                            ps3 = ps_pool.tile([P, TCH], f32, tag="ps3")
                            for ho in range(NH):
                                nc.tensor.matmul(
                                    ps1,
                                    w1b[:, ho, :],
                                    xt[:, ho, tsl],
                                    start=(ho == 0), stop=(ho == NH - 1),
                                )
                            for ho in range(NH):
                                nc.tensor.matmul(
                                    ps3,
                                    w3b[:, ho, :],
                                    xt[:, ho, tsl],
                                    start=(ho == 0), stop=(ho == NH - 1),
                                )
                            # ps1 released by ACT only; ps3 by DVE only
                            sl = hid_pool.tile([P, TCH], f32, tag="silu")
                            nc.scalar.activation(out=sl, in_=ps1, func=Silu)
                            nc.vector.tensor_mul(hb[:, tsl], sl, ps3)
                        nc.gpsimd.dma_start(
                            out=hid_dram[ft * P:(ft + 1) * P, :], in_=hb
                        )

            tc.strict_bb_all_engine_barrier()
            observe(identb)

            # ---- pass 2: out = hidden.T @ W2 ----
            with tc.tile_pool(name="oa", bufs=1) as oa_pool, \
                 tc.tile_pool(name="hld", bufs=2 * KFB) as hld_pool, \
                 tc.tile_pool(name="w2f", bufs=4) as w2f_pool, \
                 tc.tile_pool(name="w2ld", bufs=2 * KFB) as w2_pool, \
                 tc.tile_pool(name="ps2", bufs=6, space="PSUM") as ps2_pool:
                for nh in range(NHF):
                    oa = oa_pool.tile([P, NT, HH], f32, tag="oa")
                    for kfb in range(NKFB):
                        hcs = []
                        wcs = []
                        for j in range(KFB):
                            kf = kfb * KFB + j
                            hc = hld_pool.tile([P, T], bf16, tag="hc")
                            nc.sync.dma_start(
                                out=hc, in_=hid_dram[kf * P:(kf + 1) * P, :]
                            )
                            observe(hc[:, 0:P])
                            wf = w2f_pool.tile([P, HH], f32, tag="wf")
                            nc.scalar.dma_start(
                                out=wf,
                                in_=w2[kf * P:(kf + 1) * P, nh * HH:(nh + 1) * HH],
                            )
                            wc = w2_pool.tile([P, HH], bf16, tag="wc")
                            nc.scalar.activation(out=wc, in_=wf, func=Copy)
                            observe(wc[:, 0:P])
                            hcs.append(hc)
                            wcs.append(wc)
                        for mt in range(NT):
                            for nq in range(NNQ):
                                ps = ps2_pool.tile([P, NQ], f32, tag="ps2")
                                for j in range(KFB):
                                    nc.tensor.matmul(
                                        ps,
                                        hcs[j][:, mt * P:(mt + 1) * P],
                                        wcs[j][:, nq * NQ:(nq + 1) * NQ],
                                        start=(j == 0), stop=(j == KFB - 1),
                                    )
                                osl = oa[:, mt, nq * NQ:(nq + 1) * NQ]
                                if kfb == 0:
                                    nc.vector.tensor_copy(out=osl, in_=ps)
                                else:
                                    nc.vector.tensor_add(osl, osl, ps)
                    for mt in range(NT):
                        nc.gpsimd.dma_start(
                            out=out[mt * P:(mt + 1) * P, nh * HH:(nh + 1) * HH],
                            in_=oa[:, mt, :],
                        )
    nc.finalize()
    return nc


def _get_nc(T, Hd, Fd):
    key = (T, Hd, Fd)
    if key not in _built:
        _built[key] = _build(T, Hd, Fd)
    return _built[key]


def _run(x, w1, w2, w3):
    """x: (E, T, H); w1/w3: (E, H, F); w2: (E, F, H). Returns (E, T, H)."""
    from concourse.bass_utils import run_bass_kernel_spmd

    global LAST_RESULT
    Ne, T, Hd = x.shape
    Fd = w1.shape[-1]
    nc = _get_nc(T, Hd, Fd)
    in_maps = [
        {
            "x": np.ascontiguousarray(x[e], dtype=np.float32),
            "w1": np.ascontiguousarray(w1[e], dtype=np.float32),
            "w2": np.ascontiguousarray(w2[e], dtype=np.float32),
            "w3": np.ascontiguousarray(w3[e], dtype=np.float32),
        }
        for e in range(Ne)
    ]
    br = run_bass_kernel_spmd(nc, in_maps, core_ids=list(range(Ne)), trace=TRACE)
    LAST_RESULT = br
    return np.stack([br.results[e]["out"] for e in range(Ne)], axis=0)


def kernel(dispatch_input, w1, w2, w3):
    Ne, Bb, Cc, Hd = dispatch_input.shape
    xs = np.ascontiguousarray(
        np.asarray(dispatch_input, dtype=np.float32).reshape(Ne, Bb * Cc, Hd)
    )
    o = _run(xs, np.asarray(w1), np.asarray(w2), np.asarray(w3))
    return np.ascontiguousarray(o.reshape(Ne, Bb, Cc, Hd)).astype(np.float32)


# revision 3
# speedup vs baseline: 1.0535x; 1.0535x over previous
"""Mixtral expert-capacity top-2 MLP (per-expert grouped GEMM SwiGLU) on 8
NeuronCores: all matmuls in bf16 (FWL weight loads), bf16 hidden
staging, deep-buffered pass 2, HWDGE queue spreading, batched hidden writes.

Expert parallel: core e computes, for expert e,
    out_e = (silu(X_e @ W1_e) * (X_e @ W3_e)) @ W2_e
with X_e = dispatch_input[e] reshaped to (B*C, H).

Structure per core:
  phase 0: transpose X (T,H) -> XT (H,T) via PE-transpose (fp32), DVE/ACT
           evacuate PSUM with cast to bf16; XT bf16 SBUF-resident.
  pass 1:  hidden[f, t] = silu(W1.T X.T) * (W3.T X.T) in bf16 matmuls
           (weights DMA'd fp32, cast to bf16 on DVE/ACT), PSUM accumulate
           over H; hidden staged to DRAM as bf16.
  pass 2:  out[t, h] = hidden.T @ W2 (both bf16, W2 cast on ACT) with PSUM
           accumulation over F in blocks of KFB f-tiles, partials summed in
           an SBUF fp32 accumulator (one H-half at a time), then DMA'd out.

The observe() trick from V1 is kept: every DMA- or engine-produced tile PE
consumes is first "observed" by a throwaway bf16 ldweights so real matmuls
only wait on the PSUM-slot-release semaphore.
"""

import os

os.environ.setdefault("MYCRO_LOCAL_CACHE", "1")

import numpy as np

E, B, C, H, F = 8, 4, 512, 2048, 7168
P = 128

TRACE = bool(int(os.environ.get("BASS_KERNEL_TRACE", "0")))
LAST_RESULT = None
_built = {}


def _build(T, Hd, Fd):
    import concourse.bass as bass
    import concourse.mybir as mybir
    from concourse import bacc
    import concourse.tile as tile
    from concourse.masks import make_identity

    f32 = mybir.dt.float32
    bf16 = mybir.dt.bfloat16
    Silu = mybir.ActivationFunctionType.Silu
    Copy = mybir.ActivationFunctionType.Copy

    assert T % P == 0 and Hd % P == 0 and Fd % P == 0
    NT = T // P          # token tiles
    NH = Hd // P         # H contraction tiles
    NF = Fd // P         # F tiles
    TCH = min(512, T)    # pass-1 moving-dim chunk (tokens)
    NTC = T // TCH
    KFB = 8 if NF % 8 == 0 else 1   # pass-2 f-tiles per PSUM accumulation block
    NKFB = NF // KFB
    NHF = 2              # pass-2 H halves (SBUF accumulator covers T x Hd/NHF)
    assert Hd % NHF == 0
    HH = Hd // NHF
    NQ = min(512, HH)    # pass-2 moving-dim chunk (H)
    NNQ = HH // NQ

    nc = bacc.Bacc(None, target_bir_lowering=False)
    x = nc.declare_dram_parameter("x", [T, Hd], f32, isOutput=False)
    w1 = nc.declare_dram_parameter("w1", [Hd, Fd], f32, isOutput=False)
    w2 = nc.declare_dram_parameter("w2", [Fd, Hd], f32, isOutput=False)
    w3 = nc.declare_dram_parameter("w3", [Hd, Fd], f32, isOutput=False)
    out = nc.declare_dram_parameter("out", [T, Hd], f32, isOutput=True)
    hid_dram = nc.dram_tensor("hid", [Fd, T], bf16)

    w1r = w1.rearrange("(ho hi) f -> hi ho f", hi=P)   # [128, NH, Fd]
    w3r = w3.rearrange("(ho hi) f -> hi ho f", hi=P)

    with tile.TileContext(nc) as tc:
        with tc.tile_pool(name="const", bufs=1) as const_pool:
            ident = const_pool.tile([P, P], f32)
            make_identity(nc, ident)
            identb = const_pool.tile([P, P], bf16)
            nc.vector.tensor_copy(out=identb, in_=ident)

            def observe(t2d):
                """Absorb a wait into PE's clock: a throwaway bf16 ldweights
                reading only this tile.  No output -> no WAW self-wait."""
                nc.tensor.ldweights(t2d[:, 0:64].bitcast(bf16))

            with tc.tile_pool(name="xt", bufs=1) as xt_pool:
                xt = xt_pool.tile([P, NH, T], bf16)  # XT[hi, ho, t]

                # ---- phase 0: X -> XT via PE transpose, cast to bf16 ----
                with tc.tile_pool(name="xload", bufs=2) as xload_pool, \
                     tc.tile_pool(name="pst", bufs=4, space="PSUM") as pst_pool:
                    for tt in range(NT):
                        x_t = xload_pool.tile([P, Hd], f32, tag="xld")
                        nc.sync.dma_start(out=x_t, in_=x[tt * P:(tt + 1) * P, :])
                        observe(x_t[:, 0:P])
                        for ho in range(NH):
                            pt = pst_pool.tile([P, P], f32, tag="pst")
                            nc.tensor.transpose(pt, x_t[:, ho * P:(ho + 1) * P], ident)
                            osl = xt[:, ho, tt * P:(tt + 1) * P]
                            if ho % 2 == 0:
                                nc.vector.tensor_copy(out=osl, in_=pt)
                            else:
                                nc.scalar.activation(out=osl, in_=pt, func=Copy)

                tc.strict_bb_all_engine_barrier()
                observe(identb)  # re-observe post-barrier on a dep-free tile

                # ---- pass 1: hidden = silu(X@W1) * (X@W3), layout [f, t] ----
                with tc.tile_pool(name="w1c", bufs=2) as w1_pool, \
                     tc.tile_pool(name="w3c", bufs=2) as w3_pool, \
                     tc.tile_pool(name="w1b", bufs=2) as w1b_pool, \
                     tc.tile_pool(name="w3b", bufs=2) as w3b_pool, \
                     tc.tile_pool(name="hidst", bufs=4) as hid_pool, \
                     tc.tile_pool(name="ps_mm", bufs=4, space="PSUM") as ps_pool:
                    for ft in range(NF):
                        w1c = w1_pool.tile([P, NH, P], f32, tag="w1c")
                        nc.sync.dma_start(
                            out=w1c, in_=w1r[:, :, ft * P:(ft + 1) * P]
                        )
                        w1b = w1b_pool.tile([P, NH, P], bf16, tag="w1b")
                        nc.vector.tensor_copy(out=w1b, in_=w1c)
                        observe(w1b[:, 0, :])
                        w3c = w3_pool.tile([P, NH, P], f32, tag="w3c")
                        nc.scalar.dma_start(
                            out=w3c, in_=w3r[:, :, ft * P:(ft + 1) * P]
                        )
                        w3b = w3b_pool.tile([P, NH, P], bf16, tag="w3b")
                        nc.scalar.activation(out=w3b, in_=w3c, func=Copy)
                        observe(w3b[:, 0, :])
                        hb = hid_pool.tile([P, T], bf16, tag="hid")
                        for tch in range(NTC):
                            tsl = slice(tch * TCH, (tch + 1) * TCH)
                            ps1 = ps_pool.tile([P, TCH], f32, tag="ps1")
                            ps3 = ps_pool.tile([P, TCH], f32, tag="ps3")
                            for ho in range(NH):
                                nc.tensor.matmul(
                                    ps1,
                                    w1b[:, ho, :],
                                    xt[:, ho, tsl],
                                    start=(ho == 0), stop=(ho == NH - 1),
                                )
                            for ho in range(NH):
                                nc.tensor.matmul(
                                    ps3,
                                    w3b[:, ho, :],
                                    xt[:, ho, tsl],
                                    start=(ho == 0), stop=(ho == NH - 1),
                                )
                            # ps1 released by ACT only; ps3 by DVE only
                            sl = hid_pool.tile([P, TCH], f32, tag="silu")
                            nc.scalar.activation(out=sl, in_=ps1, func=Silu)
                            nc.vector.tensor_mul(hb[:, tsl], sl, ps3)
                        nc.gpsimd.dma_start(
                            out=hid_dram[ft * P:(ft + 1) * P, :], in_=hb
                        )

            tc.strict_bb_all_engine_barrier()
            observe(identb)

            # ---- pass 2: out = hidden.T @ W2 ----
            with tc.tile_pool(name="oa", bufs=1) as oa_pool, \
                 tc.tile_pool(name="hld", bufs=2 * KFB) as hld_pool, \
                 tc.tile_pool(name="w2f", bufs=4) as w2f_pool, \
                 tc.tile_pool(name="w2ld", bufs=2 * KFB) as w2_pool, \
                 tc.tile_pool(name="ps2", bufs=8, space="PSUM") as ps2_pool:
                for nh in range(NHF):
                    oa = oa_pool.tile([P, NT, HH], f32, tag="oa")
                    for kfb in range(NKFB):
                        hcs = []
                        wcs = []
                        for j in range(KFB):
                            kf = kfb * KFB + j
                            hc = hld_pool.tile([P, T], bf16, tag="hc")
                            nc.sync.dma_start(
                                out=hc, in_=hid_dram[kf * P:(kf + 1) * P, :]
                            )
                            observe(hc[:, 0:P])
                            wf = w2f_pool.tile([P, HH], f32, tag="wf")
                            nc.scalar.dma_start(
                                out=wf,
                                in_=w2[kf * P:(kf + 1) * P, nh * HH:(nh + 1) * HH],
                            )
                            wc = w2_pool.tile([P, HH], bf16, tag="wc")
                            nc.scalar.activation(out=wc, in_=wf, func=Copy)
                            observe(wc[:, 0:P])
                            hcs.append(hc)
                            wcs.append(wc)
                        for mt in range(NT):
                            for nq in range(NNQ):
                                ps = ps2_pool.tile([P, NQ], f32, tag="ps2")
                                for j in range(KFB):
                                    nc.tensor.matmul(
                                        ps,
                                        hcs[j][:, mt * P:(mt + 1) * P],
                                        wcs[j][:, nq * NQ:(nq + 1) * NQ],
                                        start=(j == 0), stop=(j == KFB - 1),
                                    )
                                osl = oa[:, mt, nq * NQ:(nq + 1) * NQ]
                                if kfb == 0:
                                    nc.vector.tensor_copy(out=osl, in_=ps)
                                else:
                                    nc.vector.tensor_add(osl, osl, ps)
                    for mt in range(NT):
                        nc.gpsimd.dma_start(
                            out=out[mt * P:(mt + 1) * P, nh * HH:(nh + 1) * HH],
                            in_=oa[:, mt, :],
                        )
    nc.finalize()
    return nc


def _get_nc(T, Hd, Fd):
    key = (T, Hd, Fd)
    if key not in _built:
        _built[key] = _build(T, Hd, Fd)
    return _built[key]


def _run(x, w1, w2, w3):
    """x: (E, T, H); w1/w3: (E, H, F); w2: (E, F, H). Returns (E, T, H)."""
    from concourse.bass_utils import run_bass_kernel_spmd

    global LAST_RESULT
    Ne, T, Hd = x.shape
    Fd = w1.shape[-1]
    nc = _get_nc(T, Hd, Fd)
    in_maps = [
        {
            "x": np.ascontiguousarray(x[e], dtype=np.float32),
            "w1": np.ascontiguousarray(w1[e], dtype=np.float32),
            "w2": np.ascontiguousarray(w2[e], dtype=np.float32),
            "w3": np.ascontiguousarray(w3[e], dtype=np.float32),
        }
        for e in range(Ne)
    ]
    br = run_bass_kernel_spmd(nc, in_maps, core_ids=list(range(Ne)), trace=TRACE)
    LAST_RESULT = br
    return np.stack([br.results[e]["out"] for e in range(Ne)], axis=0)


def kernel(dispatch_input, w1, w2, w3):
    Ne, Bb, Cc, Hd = dispatch_input.shape
    xs = np.ascontiguousarray(
        np.asarray(dispatch_input, dtype=np.float32).reshape(Ne, Bb * Cc, Hd)
    )
    o = _run(xs, np.asarray(w1), np.asarray(w2), np.asarray(w3))
    return np.ascontiguousarray(o.reshape(Ne, Bb, Cc, Hd)).astype(np.float32)
